# revision 20
# baseline (speedup 1.0000x reference)
"""ConvolutionalAttention (training branch) for Trainium2, 8 NeuronCores.

The module computes, per sample b:
    out[:, :32]  = conv13x13(x1, lk_filter) + depthwise3x3(x1, dyn_k[b])
    out[:, 32:]  = x2            (pass-through)
where dyn_k[b] comes from a tiny MLP (pool -> 1x1 -> GELU -> 1x1) on x1.

Key transformation: conv is linear in the filter, so the per-sample dynamic
depthwise 3x3 kernel is folded host-side into the center of a per-sample
13x13 dense filter.  The device then runs ONE dense 32->32 13x13 conv per
sample.  Data-parallel over batch: 2 samples per core.

Device mapping (per core, per sample), v2:
  - conv as matmul with K = 128 = (4 row-shift replicas g) x (32 in-ch),
    M = 128 = (4 output rows dy) x (32 out-ch).
  - bf16 operands: same PE stream rate as fp32r, but half the DMA bytes
    and 2x faster LDWEIGHTS (fast-weight-load path).  Accumulation stays
    fp32 in PSUM; observed end-to-end rel err ~2e-3 vs 2e-2 budget.
  - supersteps of 64 output rows: 6 PSUM accumulators [128, 512], each
    covering 32 rows x 64 cols via an overlapped rhs access pattern
    (8 quads x 64 cols).  52 weight blocks (4 ky'-chunks x 13 kx) feed
    6 back-to-back matmuls each, so walrus's redundant-LDWEIGHTS elision
    drops 5 of every 6 weight loads.
  - input/weight DMAs spread over 4 engine queues to shrink the initial
    transfer-limited head; dummy warm-up matmuls keep the PE HAM clock
    gate at 2.4 GHz by the time the real stream starts.
"""

import json

import numpy as np

import concourse.bass as bass
import concourse.mybir as mybir
import concourse.tile as tile
from concourse.bass_utils import run_bass_kernel_spmd

# ---------------------------------------------------------------------------
# Problem constants (hardcoded; kernel.py must be self-contained)
B, C, H, W = 16, 64, 192, 192
PD, SK, LK = 32, 3, 13
PAD = LK // 2                      # 6
NCORES = 8
BLOC = B // NCORES                 # 2 samples per core
PADW = W + 2 * PAD                 # 204
PADH = H + 2 * PAD                 # 204
NJ, G, DY = 4, 4, 4                # ky' chunks, row-shift replicas, rows/quad
NKX = LK                           # 13 kx shifts
SSROWS = 64                        # output rows per superstep
NSS = H // SSROWS                  # 3 supersteps per sample
RH = 2                             # 32-row halves per superstep
CS = 3                             # 64-col strips per superstep
NQ = 8                             # quads per accumulator
NCOL = 64                          # cols per strip
SROWS = 73                         # x4 rows needed per superstep (per g)
NFREE = NQ * NCOL                  # 512 matmul moving free dim (1 PSUM bank)
NWARM = 60                         # dummy warm-up matmuls (~16us span)
F32 = mybir.dt.float32
BF16 = mybir.dt.bfloat16

# ---------------------------------------------------------------------------
# Workaround: the walrus_driver in this container rejects instructions with
# more than one sync-wait command.  Post-process the BIR JSON, moving excess
# waits onto single-wait NoOps inserted right before the offending
# instruction (same engine => executes first, semantics preserved).
_orig_to_json_bytes = bass.Bass.to_json_bytes


def _split_multi_waits(m):
    import json as _json
    for f in m.get("functions", []):
        for blk in f.get("blocks", []):
            out = []
            changed = False
            last_ldw_sig = [None]
            for inst in blk.get("instructions", []):
                si = inst.get("sync_info")
                waits = (si or {}).get("on_wait") or []
                # strip sync waits off Ldweights onto NoOps so the dedup
                # below can't drop a load-bearing wait
                keep = 0 if inst["opcode"] == "Ldweights" else 1
                if len(waits) > keep:
                    changed = True
                    for k, wcond in enumerate(waits[:len(waits) - keep]):
                        out.append({
                            "debug": inst.get("debug"),
                            "engine": inst["engine"],
                            "ins": [], "outs": [],
                            "name": f"{inst['name']}.sw{k}",
                            "opcode": "NoOp",
                            "sync_info": {"on_update": [], "on_wait": [wcond]},
                            "text_hint": "split_wait",
                        })
                    si["on_wait"] = waits[len(waits) - keep:]
                # dedup: the bf16 lowering emits one prefetch Ldweights per
                # Matmult (the Matmults have ldweights=false).  Consecutive
                # identical Ldweights are idempotent -> drop repeats so each
                # weight block is loaded once per 6-matmul group.
                if inst["engine"] == "PE":
                    if inst["opcode"] == "Ldweights":
                        sig = _json.dumps(
                            [inst.get("ins"), inst.get("tile_position"),
                             inst.get("perf_mode"),
                             inst.get("is_transpose")], sort_keys=True)
                        if sig == last_ldw_sig[0]:
                            changed = True
                            ups = (si or {}).get("on_update") or []
                            if ups:
                                out.append({
                                    "debug": inst.get("debug"),
                                    "engine": inst["engine"],
                                    "ins": [], "outs": [],
                                    "name": f"{inst['name']}.dup",
                                    "opcode": "NoOp",
                                    "sync_info": {"on_update": ups,
                                                  "on_wait": []},
                                    "text_hint": "dedup_ldw",
                                })
                            continue
                        last_ldw_sig[0] = sig
                    elif inst["opcode"] not in ("Matmult", "NoOp",
                                                "EventSemaphore"):
                        last_ldw_sig[0] = None
                out.append(inst)
            if changed:
                blk["instructions"] = out
    return m


def _to_json_bytes_split(self, *a, **kw):
    data = _orig_to_json_bytes(self, *a, **kw)
    return json.dumps(_split_multi_waits(json.loads(data))).encode()


def _install_patch():
    if bass.Bass.to_json_bytes is not _to_json_bytes_split:
        bass.Bass.to_json_bytes = _to_json_bytes_split
    # NOTE: walrus's --enable-ldw-opt is left at its default (false): the
    # bf16 path lowers each matmul to a standalone prefetch Ldweights +
    # self-loading Matmult, and walrus's ldw-opt rejects standalone
    # InstLdweights outright.  The PE's 64-deep reorder window pulls the
    # prefetch Ldweights ahead of in-flight matmuls instead.


# ---------------------------------------------------------------------------
# Device kernel


def _build_nc():
    _install_patch()
    nc = bass.Bass()
    # xin is pre-replicated host-side into the exact SBUF x4 layout
    # (partition = g*32+ic, free = (s, c)) so every DMA run is a full
    # 29.8KB contiguous per-partition read
    xin = nc.declare_dram_parameter("xin", [BLOC, NSS, 128, SROWS * PADW],
                                    BF16, isOutput=False)
    wts = nc.declare_dram_parameter("wts", [BLOC, NJ, 128, NKX * 128], BF16,
                                    isOutput=False)
    yout = nc.declare_dram_parameter("yout", [BLOC, PD, H, W], BF16,
                                     isOutput=True)
    yout_ap = yout.ap()

    with tile.TileContext(nc) as tc:
        with tc.tile_pool(name="wp", bufs=1) as wp, \
             tc.tile_pool(name="xp", bufs=2) as xp, \
             tc.tile_pool(name="sp", bufs=1) as sp, \
             tc.tile_pool(name="pp", bufs=1, space="PSUM") as pp, \
             tc.tile_pool(name="op", bufs=2) as op:

            # ---- warm-up: keep PE busy during the initial DMA head so the
            # HAM clock gate reaches 2.4 GHz before the real stream starts
            warm_w = sp.tile([128, 128], BF16, tag="warmw")
            warm_x = sp.tile([128, NFREE], BF16, tag="warmx")
            nc.vector.memset(warm_w[:], 0.0)
            nc.vector.memset(warm_x[:], 0.0)
            warm_acc = pp.tile([128, NFREE], F32, tag="warm")
            for _ in range(NWARM):
                nc.tensor.matmul(warm_acc[:], warm_w[:], warm_x[:],
                                 start=True, stop=True)

            # input x4 loads: 6 partition-chunk DMAs, 2 per DMA-capable
            # queue, so each superstep's transfer runs 3-way parallel
            qs = [nc.sync, nc.scalar, nc.gpsimd]
            xcuts = [0, 21, 42, 64, 85, 106, 128]

            def load_x4(b, ss):
                x4 = xp.tile([128, SROWS * PADW + 16], BF16, tag="x4")
                for i in range(6):
                    p0, p1 = xcuts[i], xcuts[i + 1]
                    qs[i % 3].dma_start(
                        x4[p0:p1, :SROWS * PADW],
                        xin.ap()[b, ss, p0:p1, :])
                return x4

            def load_wt(b, j, nsplit=1):
                wt = wp.tile([128, NKX * 128], BF16, tag=f"wt{b}{j}")
                cuts = [128 * i // nsplit for i in range(nsplit + 1)]
                for i in range(nsplit):
                    p0, p1 = cuts[i], cuts[i + 1]
                    qs[(b * NJ + j + i) % 3].dma_start(
                        wt[p0:p1, :], wts.ap()[b, j, p0:p1, :])
                return wt

            # weight chunk (0,0) first, 3-way split: the very first matmul
            # block needs it; remaining chunks follow the first x4
            wtiles = {(0, 0): load_wt(0, 0, nsplit=3)}
            steps = [(b, ss) for b in range(BLOC) for ss in range(NSS)]
            x4_next = load_x4(*steps[0])
            for bj in [(0, 1), (0, 2), (0, 3), (1, 0), (1, 1), (1, 2),
                       (1, 3)]:
                wtiles[bj] = load_wt(*bj)

            for si, (b, ss) in enumerate(steps):
                y0 = SSROWS * ss
                x4 = x4_next
                if si + 1 < len(steps):
                    x4_next = load_x4(*steps[si + 1])
                x4a = x4[:]
                accs = [pp.tile([128, NFREE], F32, tag=f"acc{rh}{cs}",
                                name=f"acc{rh}{cs}_{si}")
                        for rh in range(RH) for cs in range(CS)]
                # weight-block-outer order: each block feeds all 6 accs
                # back-to-back so walrus's redundant-LDWEIGHTS elision
                # (--enable-ldw-opt) drops 5 of every 6 weight loads
                for j in range(NJ):
                    wt = wtiles[(b, j)]
                    for kx in range(NKX):
                        wblk = wt[:, kx * 128:(kx + 1) * 128]
                        for a, acc in enumerate(accs):
                            rh, cs = divmod(a, CS)
                            rhs = bass.AP(
                                x4a.tensor,
                                x4a.offset + (32 * rh + 4 * j) * PADW
                                + NCOL * cs + kx,
                                [list(x4a.ap[0]),
                                 [4 * PADW, NQ], [1, NCOL]])
                            nc.tensor.matmul(
                                acc[:], wblk, rhs,
                                start=(j == 0 and kx == 0),
                                stop=(j == NJ - 1 and kx == NKX - 1))
                # merge the 3 col strips of each row-half into one SBUF
                # tile so the output DMA has 768B-contiguous runs
                for rh in range(RH):
                    ot = op.tile([128, NQ * W], BF16, tag=f"ot{rh}",
                                 name=f"ot{rh}_{si}")
                    ota = ot[:]
                    for cs in range(CS):
                        acc = accs[rh * CS + cs]
                        acca = acc[:]
                        src = bass.AP(acca.tensor, acca.offset,
                                      [list(acca.ap[0]),
                                       [NCOL, NQ], [1, NCOL]])
                        dst = bass.AP(ota.tensor, ota.offset + NCOL * cs,
                                      [list(ota.ap[0]),
                                       [W, NQ], [1, NCOL]])
                        nc.vector.tensor_copy(dst, src)
                    for dy in range(DY):
                        dma_dst = bass.AP(
                            yout_ap.tensor,
                            b * PD * H * W + (y0 + 32 * rh + dy) * W,
                            [[H * W, PD], [DY * W, NQ], [1, W]])
                        # rotate across the 3 DMA queues so the final
                        # superstep's output drains in parallel
                        qs[dy % 3].dma_start(
                            dma_dst,
                            ot[32 * dy:32 * (dy + 1), :]
                            .rearrange("p (q c) -> p q c", c=W))
    return nc


_NC = None


def _get_nc():
    global _NC
    if _NC is None:
        _NC = _build_nc()
    return _NC


# ---------------------------------------------------------------------------
# Host side


def _gelu_exact(z):
    from math import erf
    return 0.5 * z * (1.0 + np.vectorize(erf)(z / np.sqrt(2.0)))


def _prepare_inputs(x, lk_filter, w1, b1, w2, b2):
    bf16 = mybir.dt.np(BF16)
    x = np.ascontiguousarray(np.asarray(x, dtype=np.float32))
    x1 = x[:, :PD]

    # dwc_proj on host (tiny): pool -> 1x1 -> exact GELU -> 1x1
    pooled = x1.mean(axis=(2, 3), dtype=np.float32)            # [B, 32]
    hid = _gelu_exact(pooled @ np.asarray(w1, np.float32).T
                      + np.asarray(b1, np.float32)).astype(np.float32)
    dyn_k = (hid @ np.asarray(w2, np.float32).T
             + np.asarray(b2, np.float32)).reshape(B, PD, SK, SK)

    # fold the per-sample depthwise 3x3 into the center of the 13x13 filter
    F = np.broadcast_to(np.asarray(lk_filter, np.float32),
                        (B, PD, PD, LK, LK)).copy()
    idx = np.arange(PD)
    ctr = PAD - SK // 2                                         # 5
    F[:, idx, idx, ctr:ctr + SK, ctr:ctr + SK] += dyn_k

    # weight blocks: wts[b, j, kx, g*32+ic, dy*32+oc] = F[b, oc, ic, 4j+g-dy, kx]
    wts = np.zeros((B, NJ, NKX, 128, 128), np.float32)
    for j in range(NJ):
        for g in range(G):
            for dy in range(DY):
                ky = 4 * j + g - dy
                if 0 <= ky < LK:
                    wts[:, j, :, g * PD:(g + 1) * PD,
                        dy * PD:(dy + 1) * PD] = \
                        F[:, :, :, ky, :].transpose(0, 3, 2, 1)
    # device layout [b, j, k, kx*128+m]: per-partition contiguous DMA runs
    wts_dev = np.ascontiguousarray(
        wts.astype(bf16).transpose(0, 1, 3, 2, 4)).reshape(
            B, NJ, 128, NKX * 128)

    xpad = np.zeros((B, PD, PADH, PADW), bf16)
    xpad[:, :, PAD:PAD + H, PAD:PAD + W] = x1.astype(bf16)
    # pre-replicate into the SBUF x4 layout: [b, ss, g*32+ic, (s, c)]
    xrep = np.empty((B, NSS, G, PD, SROWS, PADW), bf16)
    for ss in range(NSS):
        for g in range(G):
            y0 = SSROWS * ss + g
            xrep[:, ss, g] = xpad[:, :, y0:y0 + SROWS, :]
    xrep = xrep.reshape(B, NSS, 128, SROWS * PADW)

    in_maps = [{"xin": xrep[BLOC * c:BLOC * (c + 1)],
                "wts": wts_dev[BLOC * c:BLOC * (c + 1)]}
               for c in range(NCORES)]
    return x, in_maps


def _execute(in_maps, trace=False):
    nc = _get_nc()
    return run_bass_kernel_spmd(nc, in_maps, list(range(NCORES)), trace=trace)


def kernel(x, lk_filter, w1, b1, w2, b2):
    x, in_maps = _prepare_inputs(x, lk_filter, w1, b1, w2, b2)
    res = _execute(in_maps)
    out = np.empty((B, C, H, W), np.float32)
    for c in range(NCORES):
        out[BLOC * c:BLOC * (c + 1), :PD] = \
            res.results[c]["yout"].astype(np.float32)
    out[:, PD:] = x[:, PD:]
    return out


# revision 24
# speedup vs baseline: 1.0859x; 1.0859x over previous
"""ConvolutionalAttention (training branch) for Trainium2, 8 NeuronCores.

The module computes, per sample b:
    out[:, :32]  = conv13x13(x1, lk_filter) + depthwise3x3(x1, dyn_k[b])
    out[:, 32:]  = x2            (pass-through)
where dyn_k[b] comes from a tiny MLP (pool -> 1x1 -> GELU -> 1x1) on x1.

Key transformation: conv is linear in the filter, so the per-sample dynamic
depthwise 3x3 kernel is folded host-side into the center of a per-sample
13x13 dense filter.  The device then runs ONE dense 32->32 13x13 conv per
sample.  Data-parallel over batch: 2 samples per core.

Device mapping (per core, per sample), v2:
  - conv as matmul with K = 128 = (4 row-shift replicas g) x (32 in-ch),
    M = 128 = (4 output rows dy) x (32 out-ch).
  - bf16 operands: same PE stream rate as fp32r, but half the DMA bytes
    and 2x faster LDWEIGHTS (fast-weight-load path).  Accumulation stays
    fp32 in PSUM; observed end-to-end rel err ~2e-3 vs 2e-2 budget.
  - supersteps of 64 output rows: 6 PSUM accumulators [128, 512], each
    covering 32 rows x 64 cols via an overlapped rhs access pattern
    (8 quads x 64 cols).  52 weight blocks (4 ky'-chunks x 13 kx) feed
    6 back-to-back matmuls each, so walrus's redundant-LDWEIGHTS elision
    drops 5 of every 6 weight loads.
  - input/weight DMAs spread over 4 engine queues to shrink the initial
    transfer-limited head; dummy warm-up matmuls keep the PE HAM clock
    gate at 2.4 GHz by the time the real stream starts.
"""

import json

import numpy as np

import concourse.bass as bass
import concourse.mybir as mybir
import concourse.tile as tile
from concourse.bass_utils import run_bass_kernel_spmd

# ---------------------------------------------------------------------------
# Problem constants (hardcoded; kernel.py must be self-contained)
B, C, H, W = 16, 64, 192, 192
PD, SK, LK = 32, 3, 13
PAD = LK // 2                      # 6
NCORES = 8
BLOC = B // NCORES                 # 2 samples per core
PADW = W + 2 * PAD                 # 204
PADH = H + 2 * PAD                 # 204
NJ, G, DY = 4, 4, 4                # ky' chunks, row-shift replicas, rows/quad
NKX = LK                           # 13 kx shifts
SSPLAN = [(0, 32), (32, 64), (96, 64), (160, 32)]  # (y0, rows) supersteps
NSSE = len(SSPLAN)                 # small first superstep -> short DMA head;
                                   # small last superstep -> short drain tail
CS = 3                             # 64-col strips per superstep
NQ = 8                             # quads per accumulator
NCOL = 64                          # cols per strip
SROWS = 73                         # max x4 rows per superstep (per g)
NFREE = NQ * NCOL                  # 512 matmul moving free dim (1 PSUM bank)
NWARM = 30                         # dummy warm-up matmuls (~8us span)
F32 = mybir.dt.float32
BF16 = mybir.dt.bfloat16

# ---------------------------------------------------------------------------
# Workaround: the walrus_driver in this container rejects instructions with
# more than one sync-wait command.  Post-process the BIR JSON, moving excess
# waits onto single-wait NoOps inserted right before the offending
# instruction (same engine => executes first, semantics preserved).
_orig_to_json_bytes = bass.Bass.to_json_bytes


def _split_multi_waits(m):
    import json as _json
    for f in m.get("functions", []):
        for blk in f.get("blocks", []):
            out = []
            changed = False
            last_ldw_sig = [None]
            for inst in blk.get("instructions", []):
                si = inst.get("sync_info")
                waits = (si or {}).get("on_wait") or []
                # strip sync waits off Ldweights onto NoOps so the dedup
                # below can't drop a load-bearing wait
                keep = 0 if inst["opcode"] == "Ldweights" else 1
                if len(waits) > keep:
                    changed = True
                    for k, wcond in enumerate(waits[:len(waits) - keep]):
                        out.append({
                            "debug": inst.get("debug"),
                            "engine": inst["engine"],
                            "ins": [], "outs": [],
                            "name": f"{inst['name']}.sw{k}",
                            "opcode": "NoOp",
                            "sync_info": {"on_update": [], "on_wait": [wcond]},
                            "text_hint": "split_wait",
                        })
                    si["on_wait"] = waits[len(waits) - keep:]
                # dedup: the bf16 lowering emits one prefetch Ldweights per
                # Matmult (the Matmults have ldweights=false).  Consecutive
                # identical Ldweights are idempotent -> drop repeats so each
                # weight block is loaded once per 6-matmul group.
                if inst["engine"] == "PE":
                    if inst["opcode"] == "Ldweights":
                        sig = _json.dumps(
                            [inst.get("ins"), inst.get("tile_position"),
                             inst.get("perf_mode"),
                             inst.get("is_transpose")], sort_keys=True)
                        if sig == last_ldw_sig[0]:
                            changed = True
                            ups = (si or {}).get("on_update") or []
                            if ups:
                                out.append({
                                    "debug": inst.get("debug"),
                                    "engine": inst["engine"],
                                    "ins": [], "outs": [],
                                    "name": f"{inst['name']}.dup",
                                    "opcode": "NoOp",
                                    "sync_info": {"on_update": ups,
                                                  "on_wait": []},
                                    "text_hint": "dedup_ldw",
                                })
                            continue
                        last_ldw_sig[0] = sig
                    elif inst["opcode"] not in ("Matmult", "NoOp",
                                                "EventSemaphore"):
                        last_ldw_sig[0] = None
                out.append(inst)
            if changed:
                blk["instructions"] = out
    return m


def _to_json_bytes_split(self, *a, **kw):
    data = _orig_to_json_bytes(self, *a, **kw)
    return json.dumps(_split_multi_waits(json.loads(data))).encode()


def _install_patch():
    if bass.Bass.to_json_bytes is not _to_json_bytes_split:
        bass.Bass.to_json_bytes = _to_json_bytes_split
    # NOTE: walrus's --enable-ldw-opt is left at its default (false): the
    # bf16 path lowers each matmul to a standalone prefetch Ldweights +
    # self-loading Matmult, and walrus's ldw-opt rejects standalone
    # InstLdweights outright.  The PE's 64-deep reorder window pulls the
    # prefetch Ldweights ahead of in-flight matmuls instead.


# ---------------------------------------------------------------------------
# Device kernel


def _build_nc():
    _install_patch()
    nc = bass.Bass()
    # xin is pre-replicated host-side into the exact SBUF x4 layout
    # (partition = g*32+ic, free = (s, c)) so every DMA run is a full
    # contiguous per-partition read
    xin = nc.declare_dram_parameter("xin", [BLOC, NSSE, 128, SROWS * PADW],
                                    BF16, isOutput=False)
    wts = nc.declare_dram_parameter("wts", [BLOC, NJ, 128, NKX * 128], BF16,
                                    isOutput=False)
    yout = nc.declare_dram_parameter("yout", [BLOC, PD, H, W], BF16,
                                     isOutput=True)
    yout_ap = yout.ap()

    with tile.TileContext(nc) as tc:
        with tc.tile_pool(name="wp", bufs=1) as wp, \
             tc.tile_pool(name="xp", bufs=2) as xp, \
             tc.tile_pool(name="sp", bufs=1) as sp, \
             tc.tile_pool(name="pp", bufs=1, space="PSUM") as pp, \
             tc.tile_pool(name="op", bufs=2) as op:

            # ---- warm-up: keep PE busy during the initial DMA head so the
            # HAM clock gate reaches 2.4 GHz before the real stream starts
            warm_w = sp.tile([128, 128], BF16, tag="warmw")
            warm_x = sp.tile([128, NFREE], BF16, tag="warmx")
            nc.vector.memset(warm_w[:], 0.0)
            nc.vector.memset(warm_x[:], 0.0)
            warm_acc = pp.tile([128, NFREE], F32, tag="warm")
            for _ in range(NWARM):
                nc.tensor.matmul(warm_acc[:], warm_w[:], warm_x[:],
                                 start=True, stop=True)

            # input x4 loads: one 32-partition-aligned DMA per row-shift
            # replica g, spread over the 3 DMA-capable queues (partition
            # slices must stay 32-aligned: unaligned chunks transfer ~3x
            # slower and their SBUF writes contend with PE reads)
            qs = [nc.sync, nc.scalar, nc.gpsimd]
            xqs = [nc.sync, nc.scalar, nc.gpsimd, nc.scalar]

            def load_x4(b, ssi):
                rows = SSPLAN[ssi][1]
                srows = rows + 9
                x4 = xp.tile([128, SROWS * PADW + 16], BF16, tag="x4")
                for g in range(G):
                    xqs[g].dma_start(
                        x4[32 * g:32 * (g + 1), :srows * PADW],
                        xin.ap()[b, ssi, 32 * g:32 * (g + 1),
                                 :srows * PADW])
                return x4

            def load_wt(b, j, nsplit=1):
                wt = wp.tile([128, NKX * 128], BF16, tag=f"wt{b}{j}")
                cuts = [0, 32, 64, 96, 128]
                step = 4 // nsplit
                for i in range(nsplit):
                    p0, p1 = cuts[i * step], cuts[(i + 1) * step]
                    qs[(b * NJ + j + i) % 3].dma_start(
                        wt[p0:p1, :], wts.ap()[b, j, p0:p1, :])
                return wt

            # weight chunk (0,0) first, split over queues: the very first
            # matmul block needs it; remaining chunks follow the first x4
            wtiles = {(0, 0): load_wt(0, 0, nsplit=2)}
            steps = [(b, ssi) for b in range(BLOC) for ssi in range(NSSE)]
            x4_next = load_x4(*steps[0])
            for bj in [(0, 1), (0, 2), (0, 3), (1, 0), (1, 1), (1, 2),
                       (1, 3)]:
                wtiles[bj] = load_wt(*bj)

            for si, (b, ssi) in enumerate(steps):
                y0, rows = SSPLAN[ssi]
                nrh = rows // 32
                x4 = x4_next
                if si + 1 < len(steps):
                    x4_next = load_x4(*steps[si + 1])
                x4a = x4[:]
                accs = [pp.tile([128, NFREE], F32, tag=f"acc{rh}{cs}",
                                name=f"acc{rh}{cs}_{si}")
                        for rh in range(nrh) for cs in range(CS)]
                # weight-block-outer order: each block feeds all accs
                # back-to-back; the BIR postprocess dedupes the repeated
                # prefetch Ldweights so each block is loaded once
                for j in range(NJ):
                    wt = wtiles[(b, j)]
                    for kx in range(NKX):
                        wblk = wt[:, kx * 128:(kx + 1) * 128]
                        for a, acc in enumerate(accs):
                            rh, cs = divmod(a, CS)
                            rhs = bass.AP(
                                x4a.tensor,
                                x4a.offset + (32 * rh + 4 * j) * PADW
                                + NCOL * cs + kx,
                                [list(x4a.ap[0]),
                                 [4 * PADW, NQ], [1, NCOL]])
                            nc.tensor.matmul(
                                acc[:], wblk, rhs,
                                start=(j == 0 and kx == 0),
                                stop=(j == NJ - 1 and kx == NKX - 1))
                # merge the 3 col strips of each row-half into one SBUF
                # tile so the output DMA has 384B-contiguous runs
                for rh in range(nrh):
                    ot = op.tile([128, NQ * W], BF16, tag=f"ot{rh}",
                                 name=f"ot{rh}_{si}")
                    ota = ot[:]
                    for cs in range(CS):
                        acc = accs[rh * CS + cs]
                        acca = acc[:]
                        src = bass.AP(acca.tensor, acca.offset,
                                      [list(acca.ap[0]),
                                       [NCOL, NQ], [1, NCOL]])
                        dst = bass.AP(ota.tensor, ota.offset + NCOL * cs,
                                      [list(ota.ap[0]),
                                       [W, NQ], [1, NCOL]])
                        nc.vector.tensor_copy(dst, src)
                    for dy in range(DY):
                        dma_dst = bass.AP(
                            yout_ap.tensor,
                            b * PD * H * W + (y0 + 32 * rh + dy) * W,
                            [[H * W, PD], [DY * W, NQ], [1, W]])
                        # rotate across the 3 DMA queues so the final
                        # superstep's output drains in parallel
                        qs[dy % 3].dma_start(
                            dma_dst,
                            ot[32 * dy:32 * (dy + 1), :]
                            .rearrange("p (q c) -> p q c", c=W))
    return nc


_NC = None


def _get_nc():
    global _NC
    if _NC is None:
        _NC = _build_nc()
    return _NC


# ---------------------------------------------------------------------------
# Host side


def _gelu_exact(z):
    from math import erf
    return 0.5 * z * (1.0 + np.vectorize(erf)(z / np.sqrt(2.0)))


def _prepare_inputs(x, lk_filter, w1, b1, w2, b2):
    bf16 = mybir.dt.np(BF16)
    x = np.ascontiguousarray(np.asarray(x, dtype=np.float32))
    x1 = x[:, :PD]

    # dwc_proj on host (tiny): pool -> 1x1 -> exact GELU -> 1x1
    pooled = x1.mean(axis=(2, 3), dtype=np.float32)            # [B, 32]
    hid = _gelu_exact(pooled @ np.asarray(w1, np.float32).T
                      + np.asarray(b1, np.float32)).astype(np.float32)
    dyn_k = (hid @ np.asarray(w2, np.float32).T
             + np.asarray(b2, np.float32)).reshape(B, PD, SK, SK)

    # fold the per-sample depthwise 3x3 into the center of the 13x13 filter
    F = np.broadcast_to(np.asarray(lk_filter, np.float32),
                        (B, PD, PD, LK, LK)).copy()
    idx = np.arange(PD)
    ctr = PAD - SK // 2                                         # 5
    F[:, idx, idx, ctr:ctr + SK, ctr:ctr + SK] += dyn_k

    # weight blocks: wts[b, j, kx, g*32+ic, dy*32+oc] = F[b, oc, ic, 4j+g-dy, kx]
    wts = np.zeros((B, NJ, NKX, 128, 128), np.float32)
    for j in range(NJ):
        for g in range(G):
            for dy in range(DY):
                ky = 4 * j + g - dy
                if 0 <= ky < LK:
                    wts[:, j, :, g * PD:(g + 1) * PD,
                        dy * PD:(dy + 1) * PD] = \
                        F[:, :, :, ky, :].transpose(0, 3, 2, 1)
    # device layout [b, j, k, kx*128+m]: per-partition contiguous DMA runs
    wts_dev = np.ascontiguousarray(
        wts.astype(bf16).transpose(0, 1, 3, 2, 4)).reshape(
            B, NJ, 128, NKX * 128)

    xpad = np.zeros((B, PD, PADH, PADW), bf16)
    xpad[:, :, PAD:PAD + H, PAD:PAD + W] = x1.astype(bf16)
    # pre-replicate into the SBUF x4 layout: [b, ssi, g*32+ic, (s, c)]
    xrep = np.zeros((B, NSSE, G, PD, SROWS, PADW), bf16)
    for ssi, (y0, rows) in enumerate(SSPLAN):
        srows = rows + 9
        for g in range(G):
            xrep[:, ssi, g, :, :srows] = \
                xpad[:, :, y0 + g:y0 + g + srows, :]
    xrep = xrep.reshape(B, NSSE, 128, SROWS * PADW)

    in_maps = [{"xin": xrep[BLOC * c:BLOC * (c + 1)],
                "wts": wts_dev[BLOC * c:BLOC * (c + 1)]}
               for c in range(NCORES)]
    return x, in_maps


def _execute(in_maps, trace=False):
    nc = _get_nc()
    return run_bass_kernel_spmd(nc, in_maps, list(range(NCORES)), trace=trace)


def kernel(x, lk_filter, w1, b1, w2, b2):
    x, in_maps = _prepare_inputs(x, lk_filter, w1, b1, w2, b2)
    res = _execute(in_maps)
    out = np.empty((B, C, H, W), np.float32)
    for c in range(NCORES):
        out[BLOC * c:BLOC * (c + 1), :PD] = \
            res.results[c]["yout"].astype(np.float32)
    out[:, PD:] = x[:, PD:]
    return out


# revision 25
# speedup vs baseline: 1.2885x; 1.1865x over previous
"""ConvolutionalAttention (training branch) for Trainium2, 8 NeuronCores.

The module computes, per sample b:
    out[:, :32]  = conv13x13(x1, lk_filter) + depthwise3x3(x1, dyn_k[b])
    out[:, 32:]  = x2            (pass-through)
where dyn_k[b] comes from a tiny MLP (pool -> 1x1 -> GELU -> 1x1) on x1.

Key transformation: conv is linear in the filter, so the per-sample dynamic
depthwise 3x3 kernel is folded host-side into the center of a per-sample
13x13 dense filter.  The device then runs ONE dense 32->32 13x13 conv per
sample.  Data-parallel over batch: 2 samples per core.

Device mapping (per core, per sample), v2:
  - conv as matmul with K = 128 = (4 row-shift replicas g) x (32 in-ch),
    M = 128 = (4 output rows dy) x (32 out-ch).
  - bf16 operands: same PE stream rate as fp32r, but half the DMA bytes
    and 2x faster LDWEIGHTS (fast-weight-load path).  Accumulation stays
    fp32 in PSUM; observed end-to-end rel err ~2e-3 vs 2e-2 budget.
  - supersteps of 64 output rows: 6 PSUM accumulators [128, 512], each
    covering 32 rows x 64 cols via an overlapped rhs access pattern
    (8 quads x 64 cols).  52 weight blocks (4 ky'-chunks x 13 kx) feed
    6 back-to-back matmuls each, so walrus's redundant-LDWEIGHTS elision
    drops 5 of every 6 weight loads.
  - input/weight DMAs spread over 4 engine queues to shrink the initial
    transfer-limited head; dummy warm-up matmuls keep the PE HAM clock
    gate at 2.4 GHz by the time the real stream starts.
"""

import json

import numpy as np

import concourse.bass as bass
import concourse.mybir as mybir
import concourse.tile as tile
from concourse.bass_utils import run_bass_kernel_spmd

# ---------------------------------------------------------------------------
# Problem constants (hardcoded; kernel.py must be self-contained)
B, C, H, W = 16, 64, 192, 192
PD, SK, LK = 32, 3, 13
PAD = LK // 2                      # 6
NCORES = 8
BLOC = B // NCORES                 # 2 samples per core
PADW = W + 2 * PAD                 # 204
PADH = H + 2 * PAD                 # 204
NJ, G, DY = 4, 4, 4                # ky' chunks, row-shift replicas, rows/quad
NKX = LK                           # 13 kx shifts
SSPLAN = [(0, 32), (32, 64), (96, 64), (160, 32)]  # (y0, rows) supersteps
NSSE = len(SSPLAN)                 # small first superstep -> short DMA head;
                                   # small last superstep -> short drain tail
CS = 3                             # 64-col strips per superstep
NQ = 8                             # quads per accumulator
NCOL = 64                          # cols per strip
SROWS = 73                         # max x4 rows per superstep (per g)
NFREE = NQ * NCOL                  # 512 matmul moving free dim (1 PSUM bank)
NWARM = 30                         # dummy warm-up matmuls (~8us span)
F32 = mybir.dt.float32
BF16 = mybir.dt.bfloat16

# ---------------------------------------------------------------------------
# Workaround: the walrus_driver in this container rejects instructions with
# more than one sync-wait command.  Post-process the BIR JSON, moving excess
# waits onto single-wait NoOps inserted right before the offending
# instruction (same engine => executes first, semantics preserved).
_orig_to_json_bytes = bass.Bass.to_json_bytes


def _split_multi_waits(m):
    import json as _json
    for f in m.get("functions", []):
        for blk in f.get("blocks", []):
            out = []
            changed = False
            last_ldw_sig = [None]
            for inst in blk.get("instructions", []):
                si = inst.get("sync_info")
                waits = (si or {}).get("on_wait") or []
                # strip sync waits off Ldweights onto NoOps so the dedup
                # below can't drop a load-bearing wait
                keep = 0 if inst["opcode"] == "Ldweights" else 1
                if len(waits) > keep:
                    changed = True
                    for k, wcond in enumerate(waits[:len(waits) - keep]):
                        out.append({
                            "debug": inst.get("debug"),
                            "engine": inst["engine"],
                            "ins": [], "outs": [],
                            "name": f"{inst['name']}.sw{k}",
                            "opcode": "NoOp",
                            "sync_info": {"on_update": [], "on_wait": [wcond]},
                            "text_hint": "split_wait",
                        })
                    si["on_wait"] = waits[len(waits) - keep:]
                # dedup: the bf16 lowering emits one prefetch Ldweights per
                # Matmult (the Matmults have ldweights=false).  Consecutive
                # identical Ldweights are idempotent -> drop repeats so each
                # weight block is loaded once per 6-matmul group.
                if inst["engine"] == "PE":
                    if inst["opcode"] == "Ldweights":
                        sig = _json.dumps(
                            [inst.get("ins"), inst.get("tile_position"),
                             inst.get("perf_mode"),
                             inst.get("is_transpose")], sort_keys=True)
                        if sig == last_ldw_sig[0]:
                            changed = True
                            ups = (si or {}).get("on_update") or []
                            if ups:
                                out.append({
                                    "debug": inst.get("debug"),
                                    "engine": inst["engine"],
                                    "ins": [], "outs": [],
                                    "name": f"{inst['name']}.dup",
                                    "opcode": "NoOp",
                                    "sync_info": {"on_update": ups,
                                                  "on_wait": []},
                                    "text_hint": "dedup_ldw",
                                })
                            continue
                        last_ldw_sig[0] = sig
                    elif inst["opcode"] not in ("Matmult", "NoOp",
                                                "EventSemaphore"):
                        last_ldw_sig[0] = None
                out.append(inst)
            if changed:
                blk["instructions"] = out
    return m


def _to_json_bytes_split(self, *a, **kw):
    data = _orig_to_json_bytes(self, *a, **kw)
    return json.dumps(_split_multi_waits(json.loads(data))).encode()


def _install_patch():
    if bass.Bass.to_json_bytes is not _to_json_bytes_split:
        bass.Bass.to_json_bytes = _to_json_bytes_split
    # NOTE: walrus's --enable-ldw-opt is left at its default (false): the
    # bf16 path lowers each matmul to a standalone prefetch Ldweights +
    # self-loading Matmult, and walrus's ldw-opt rejects standalone
    # InstLdweights outright.  The PE's 64-deep reorder window pulls the
    # prefetch Ldweights ahead of in-flight matmuls instead.


# ---------------------------------------------------------------------------
# Device kernel


def _build_nc():
    _install_patch()
    nc = bass.Bass()
    # xin is pre-replicated host-side into the exact SBUF x4 layout
    # (partition = g*32+ic, free = (s, c)) so every DMA run is a full
    # contiguous per-partition read
    xin = nc.declare_dram_parameter("xin", [BLOC, NSSE, 128, SROWS * PADW],
                                    BF16, isOutput=False)
    wts = nc.declare_dram_parameter("wts", [BLOC, NJ, 128, NKX * 128], BF16,
                                    isOutput=False)
    yout = nc.declare_dram_parameter("yout", [BLOC, PD, H, W], F32,
                                     isOutput=True)
    yout_ap = yout.ap()

    with tile.TileContext(nc) as tc:
        with tc.tile_pool(name="wp", bufs=1) as wp, \
             tc.tile_pool(name="xp", bufs=2) as xp, \
             tc.tile_pool(name="sp", bufs=1) as sp, \
             tc.tile_pool(name="pp", bufs=1, space="PSUM") as pp, \
             tc.tile_pool(name="op", bufs=2) as op:

            # ---- warm-up: keep PE busy during the initial DMA head so the
            # HAM clock gate reaches 2.4 GHz before the real stream starts
            warm_w = sp.tile([128, 128], BF16, tag="warmw")
            warm_x = sp.tile([128, NFREE], BF16, tag="warmx")
            nc.vector.memset(warm_w[:], 0.0)
            nc.vector.memset(warm_x[:], 0.0)
            warm_acc = pp.tile([128, NFREE], F32, tag="warm")
            for _ in range(NWARM):
                nc.tensor.matmul(warm_acc[:], warm_w[:], warm_x[:],
                                 start=True, stop=True)

            # input x4 loads: one 32-partition-aligned DMA per row-shift
            # replica g, spread over the 3 DMA-capable queues (partition
            # slices must stay 32-aligned: unaligned chunks transfer ~3x
            # slower and their SBUF writes contend with PE reads)
            qs = [nc.sync, nc.scalar, nc.gpsimd]
            xqs = [nc.sync, nc.scalar, nc.gpsimd, nc.scalar]

            def load_x4(b, ssi):
                rows = SSPLAN[ssi][1]
                srows = rows + 9
                x4 = xp.tile([128, SROWS * PADW + 16], BF16, tag="x4")
                for g in range(G):
                    xqs[g].dma_start(
                        x4[32 * g:32 * (g + 1), :srows * PADW],
                        xin.ap()[b, ssi, 32 * g:32 * (g + 1),
                                 :srows * PADW])
                return x4

            def load_wt(b, j, nsplit=1):
                wt = wp.tile([128, NKX * 128], BF16, tag=f"wt{b}{j}")
                cuts = [0, 32, 64, 96, 128]
                step = 4 // nsplit
                for i in range(nsplit):
                    p0, p1 = cuts[i * step], cuts[(i + 1) * step]
                    qs[(b * NJ + j + i) % 3].dma_start(
                        wt[p0:p1, :], wts.ap()[b, j, p0:p1, :])
                return wt

            # weight chunk (0,0) first, split over queues: the very first
            # matmul block needs it; remaining chunks follow the first x4
            wtiles = {(0, 0): load_wt(0, 0, nsplit=2)}
            steps = [(b, ssi) for b in range(BLOC) for ssi in range(NSSE)]
            x4_next = load_x4(*steps[0])
            for bj in [(0, 1), (0, 2), (0, 3), (1, 0), (1, 1), (1, 2),
                       (1, 3)]:
                wtiles[bj] = load_wt(*bj)

            for si, (b, ssi) in enumerate(steps):
                y0, rows = SSPLAN[ssi]
                nrh = rows // 32
                x4 = x4_next
                if si + 1 < len(steps):
                    x4_next = load_x4(*steps[si + 1])
                x4a = x4[:]
                accs = [pp.tile([128, NFREE], F32, tag=f"acc{rh}{cs}",
                                name=f"acc{rh}{cs}_{si}")
                        for rh in range(nrh) for cs in range(CS)]
                # weight-block-outer order: each block feeds all accs
                # back-to-back; the BIR postprocess dedupes the repeated
                # prefetch Ldweights so each block is loaded once
                for j in range(NJ):
                    wt = wtiles[(b, j)]
                    for kx in range(NKX):
                        wblk = wt[:, kx * 128:(kx + 1) * 128]
                        for a, acc in enumerate(accs):
                            rh, cs = divmod(a, CS)
                            rhs = bass.AP(
                                x4a.tensor,
                                x4a.offset + (32 * rh + 4 * j) * PADW
                                + NCOL * cs + kx,
                                [list(x4a.ap[0]),
                                 [4 * PADW, NQ], [1, NCOL]])
                            nc.tensor.matmul(
                                acc[:], wblk, rhs,
                                start=(j == 0 and kx == 0),
                                stop=(j == NJ - 1 and kx == NKX - 1))
                # merge the 3 col strips of each row-half into one SBUF
                # tile so the output DMA has 384B-contiguous runs
                for rh in range(nrh):
                    ot = op.tile([128, NQ * W], F32, tag=f"ot{rh}",
                                 name=f"ot{rh}_{si}")
                    ota = ot[:]
                    for cs in range(CS):
                        acc = accs[rh * CS + cs]
                        acca = acc[:]
                        src = bass.AP(acca.tensor, acca.offset,
                                      [list(acca.ap[0]),
                                       [NCOL, NQ], [1, NCOL]])
                        dst = bass.AP(ota.tensor, ota.offset + NCOL * cs,
                                      [list(ota.ap[0]),
                                       [W, NQ], [1, NCOL]])
                        nc.vector.tensor_copy(dst, src)
                    for dy in range(DY):
                        dma_dst = bass.AP(
                            yout_ap.tensor,
                            b * PD * H * W + (y0 + 32 * rh + dy) * W,
                            [[H * W, PD], [DY * W, NQ], [1, W]])
                        # rotate across the 3 DMA queues so the final
                        # superstep's output drains in parallel
                        nc.gpsimd.dma_start(
                            dma_dst,
                            ot[32 * dy:32 * (dy + 1), :]
                            .rearrange("p (q c) -> p q c", c=W))
    return nc


_NC = None


def _get_nc():
    global _NC
    if _NC is None:
        _NC = _build_nc()
    return _NC


# ---------------------------------------------------------------------------
# Host side


def _gelu_exact(z):
    from math import erf
    return 0.5 * z * (1.0 + np.vectorize(erf)(z / np.sqrt(2.0)))


def _prepare_inputs(x, lk_filter, w1, b1, w2, b2):
    bf16 = mybir.dt.np(BF16)
    x = np.ascontiguousarray(np.asarray(x, dtype=np.float32))
    x1 = x[:, :PD]

    # dwc_proj on host (tiny): pool -> 1x1 -> exact GELU -> 1x1
    pooled = x1.mean(axis=(2, 3), dtype=np.float32)            # [B, 32]
    hid = _gelu_exact(pooled @ np.asarray(w1, np.float32).T
                      + np.asarray(b1, np.float32)).astype(np.float32)
    dyn_k = (hid @ np.asarray(w2, np.float32).T
             + np.asarray(b2, np.float32)).reshape(B, PD, SK, SK)

    # fold the per-sample depthwise 3x3 into the center of the 13x13 filter
    F = np.broadcast_to(np.asarray(lk_filter, np.float32),
                        (B, PD, PD, LK, LK)).copy()
    idx = np.arange(PD)
    ctr = PAD - SK // 2                                         # 5
    F[:, idx, idx, ctr:ctr + SK, ctr:ctr + SK] += dyn_k

    # weight blocks: wts[b, j, kx, g*32+ic, dy*32+oc] = F[b, oc, ic, 4j+g-dy, kx]
    wts = np.zeros((B, NJ, NKX, 128, 128), np.float32)
    for j in range(NJ):
        for g in range(G):
            for dy in range(DY):
                ky = 4 * j + g - dy
                if 0 <= ky < LK:
                    wts[:, j, :, g * PD:(g + 1) * PD,
                        dy * PD:(dy + 1) * PD] = \
                        F[:, :, :, ky, :].transpose(0, 3, 2, 1)
    # device layout [b, j, k, kx*128+m]: per-partition contiguous DMA runs
    wts_dev = np.ascontiguousarray(
        wts.astype(bf16).transpose(0, 1, 3, 2, 4)).reshape(
            B, NJ, 128, NKX * 128)

    xpad = np.zeros((B, PD, PADH, PADW), bf16)
    xpad[:, :, PAD:PAD + H, PAD:PAD + W] = x1.astype(bf16)
    # pre-replicate into the SBUF x4 layout: [b, ssi, g*32+ic, (s, c)]
    xrep = np.zeros((B, NSSE, G, PD, SROWS, PADW), bf16)
    for ssi, (y0, rows) in enumerate(SSPLAN):
        srows = rows + 9
        for g in range(G):
            xrep[:, ssi, g, :, :srows] = \
                xpad[:, :, y0 + g:y0 + g + srows, :]
    xrep = xrep.reshape(B, NSSE, 128, SROWS * PADW)

    in_maps = [{"xin": xrep[BLOC * c:BLOC * (c + 1)],
                "wts": wts_dev[BLOC * c:BLOC * (c + 1)]}
               for c in range(NCORES)]
    return x, in_maps


def _execute(in_maps, trace=False):
    nc = _get_nc()
    return run_bass_kernel_spmd(nc, in_maps, list(range(NCORES)), trace=trace)


def kernel(x, lk_filter, w1, b1, w2, b2):
    x, in_maps = _prepare_inputs(x, lk_filter, w1, b1, w2, b2)
    res = _execute(in_maps)
    out = np.empty((B, C, H, W), np.float32)
    for c in range(NCORES):
        out[BLOC * c:BLOC * (c + 1), :PD] = res.results[c]["yout"]
    out[:, PD:] = x[:, PD:]
    return out


# revision 29
# speedup vs baseline: 1.3029x; 1.0112x over previous
"""ConvolutionalAttention (training branch) for Trainium2, 8 NeuronCores.

The module computes, per sample b:
    out[:, :32]  = conv13x13(x1, lk_filter) + depthwise3x3(x1, dyn_k[b])
    out[:, 32:]  = x2            (pass-through)
where dyn_k[b] comes from a tiny MLP (pool -> 1x1 -> GELU -> 1x1) on x1.

Key transformation: conv is linear in the filter, so the per-sample dynamic
depthwise 3x3 kernel is folded host-side into the center of a per-sample
13x13 dense filter.  The device then runs ONE dense 32->32 13x13 conv per
sample.  Data-parallel over batch: 2 samples per core.

Device mapping (per core, per sample), v2:
  - conv as matmul with K = 128 = (4 row-shift replicas g) x (32 in-ch),
    M = 128 = (4 output rows dy) x (32 out-ch).
  - bf16 operands: same PE stream rate as fp32r, but half the DMA bytes
    and 2x faster LDWEIGHTS (fast-weight-load path).  Accumulation stays
    fp32 in PSUM; observed end-to-end rel err ~2e-3 vs 2e-2 budget.
  - supersteps of 64 output rows: 6 PSUM accumulators [128, 512], each
    covering 32 rows x 64 cols via an overlapped rhs access pattern
    (8 quads x 64 cols).  52 weight blocks (4 ky'-chunks x 13 kx) feed
    6 back-to-back matmuls each, so walrus's redundant-LDWEIGHTS elision
    drops 5 of every 6 weight loads.
  - input/weight DMAs spread over 4 engine queues to shrink the initial
    transfer-limited head; dummy warm-up matmuls keep the PE HAM clock
    gate at 2.4 GHz by the time the real stream starts.
"""

import json

import numpy as np

import concourse.bass as bass
import concourse.mybir as mybir
import concourse.tile as tile
from concourse.bass_utils import run_bass_kernel_spmd

# ---------------------------------------------------------------------------
# Problem constants (hardcoded; kernel.py must be self-contained)
B, C, H, W = 16, 64, 192, 192
PD, SK, LK = 32, 3, 13
PAD = LK // 2                      # 6
NCORES = 8
BLOC = B // NCORES                 # 2 samples per core
PADW = W + 2 * PAD                 # 204
PADH = H + 2 * PAD                 # 204
NJ, G, DY = 4, 4, 4                # ky' chunks, row-shift replicas, rows/quad
NKX = LK                           # 13 kx shifts
SSPLAN = [(0, 32), (32, 64), (96, 64), (160, 32)]  # (y0, rows) supersteps
NSSE = len(SSPLAN)                 # small first superstep -> short DMA head;
                                   # small last superstep -> short drain tail
CS = 3                             # 64-col strips per superstep
NQ = 8                             # quads per accumulator
NCOL = 64                          # cols per strip
SROWS = 73                         # max x4 rows per superstep (per g)
NFREE = NQ * NCOL                  # 512 matmul moving free dim (1 PSUM bank)
NWARM = 55                         # dummy warm-up matmuls (~14us span)
F32 = mybir.dt.float32
BF16 = mybir.dt.bfloat16

# ---------------------------------------------------------------------------
# Workaround: the walrus_driver in this container rejects instructions with
# more than one sync-wait command.  Post-process the BIR JSON, moving excess
# waits onto single-wait NoOps inserted right before the offending
# instruction (same engine => executes first, semantics preserved).
_orig_to_json_bytes = bass.Bass.to_json_bytes


def _split_multi_waits(m):
    import json as _json
    for f in m.get("functions", []):
        for blk in f.get("blocks", []):
            out = []
            changed = False
            last_ldw_sig = [None]
            for inst in blk.get("instructions", []):
                si = inst.get("sync_info")
                waits = (si or {}).get("on_wait") or []
                # strip sync waits off Ldweights onto NoOps so the dedup
                # below can't drop a load-bearing wait
                keep = 0 if inst["opcode"] == "Ldweights" else 1
                if len(waits) > keep:
                    changed = True
                    for k, wcond in enumerate(waits[:len(waits) - keep]):
                        out.append({
                            "debug": inst.get("debug"),
                            "engine": inst["engine"],
                            "ins": [], "outs": [],
                            "name": f"{inst['name']}.sw{k}",
                            "opcode": "NoOp",
                            "sync_info": {"on_update": [], "on_wait": [wcond]},
                            "text_hint": "split_wait",
                        })
                    si["on_wait"] = waits[len(waits) - keep:]
                # dedup: the bf16 lowering emits one prefetch Ldweights per
                # Matmult (the Matmults have ldweights=false).  Consecutive
                # identical Ldweights are idempotent -> drop repeats so each
                # weight block is loaded once per 6-matmul group.
                if inst["engine"] == "PE":
                    if inst["opcode"] == "Ldweights":
                        sig = _json.dumps(
                            [inst.get("ins"), inst.get("tile_position"),
                             inst.get("perf_mode"),
                             inst.get("is_transpose")], sort_keys=True)
                        if sig == last_ldw_sig[0]:
                            changed = True
                            ups = (si or {}).get("on_update") or []
                            if ups:
                                out.append({
                                    "debug": inst.get("debug"),
                                    "engine": inst["engine"],
                                    "ins": [], "outs": [],
                                    "name": f"{inst['name']}.dup",
                                    "opcode": "NoOp",
                                    "sync_info": {"on_update": ups,
                                                  "on_wait": []},
                                    "text_hint": "dedup_ldw",
                                })
                            continue
                        last_ldw_sig[0] = sig
                    elif inst["opcode"] not in ("Matmult", "NoOp",
                                                "EventSemaphore"):
                        last_ldw_sig[0] = None
                out.append(inst)
            if changed:
                blk["instructions"] = out
    return m


def _to_json_bytes_split(self, *a, **kw):
    data = _orig_to_json_bytes(self, *a, **kw)
    return json.dumps(_split_multi_waits(json.loads(data))).encode()


def _install_patch():
    if bass.Bass.to_json_bytes is not _to_json_bytes_split:
        bass.Bass.to_json_bytes = _to_json_bytes_split
    # NOTE: walrus's --enable-ldw-opt is left at its default (false): the
    # bf16 path lowers each matmul to a standalone prefetch Ldweights +
    # self-loading Matmult, and walrus's ldw-opt rejects standalone
    # InstLdweights outright.  The PE's 64-deep reorder window pulls the
    # prefetch Ldweights ahead of in-flight matmuls instead.


# ---------------------------------------------------------------------------
# Device kernel


def _build_nc():
    _install_patch()
    nc = bass.Bass()
    # xin is pre-replicated host-side into the exact SBUF x4 layout
    # (partition = g*32+ic, free = (s, c)) so every DMA run is a full
    # contiguous per-partition read
    xin = nc.declare_dram_parameter("xin", [BLOC, NSSE, 128, SROWS * PADW],
                                    BF16, isOutput=False)
    wts = nc.declare_dram_parameter("wts", [BLOC, NJ, 128, NKX * 128], BF16,
                                    isOutput=False)
    yout = nc.declare_dram_parameter("yout", [BLOC, PD, H, W], F32,
                                     isOutput=True)
    yout_ap = yout.ap()

    with tile.TileContext(nc) as tc:
        with tc.tile_pool(name="wp", bufs=1) as wp, \
             tc.tile_pool(name="xp", bufs=2) as xp, \
             tc.tile_pool(name="sp", bufs=1) as sp, \
             tc.tile_pool(name="pp", bufs=1, space="PSUM") as pp, \
             tc.tile_pool(name="op", bufs=2) as op:

            # ---- warm-up: keep PE busy during the initial DMA head so the
            # HAM clock gate reaches 2.4 GHz before the real stream starts
            warm_w = sp.tile([128, 128], BF16, tag="warmw")
            warm_x = sp.tile([128, NFREE], BF16, tag="warmx")
            nc.vector.memset(warm_w[:], 0.0)
            nc.vector.memset(warm_x[:], 0.0)
            warm_acc = pp.tile([128, NFREE], F32, tag="warm")
            for _ in range(NWARM):
                nc.tensor.matmul(warm_acc[:], warm_w[:], warm_x[:],
                                 start=True, stop=True)

            # input x4 loads: one 32-partition-aligned DMA per row-shift
            # replica g, spread over the 3 DMA-capable queues (partition
            # slices must stay 32-aligned: unaligned chunks transfer ~3x
            # slower and their SBUF writes contend with PE reads)
            qs = [nc.sync, nc.scalar, nc.gpsimd]
            xqs = [nc.sync, nc.scalar, nc.gpsimd, nc.scalar]

            def load_x4(b, ssi, split=False):
                rows = SSPLAN[ssi][1]
                srows = rows + 9
                x4 = xp.tile([128, SROWS * PADW + 16], BF16, tag="x4")
                if split:
                    # first load: 8 half-row DMAs balanced over the 3
                    # queues so the head transfer finishes sooner
                    half = (srows // 2) * PADW
                    for i in range(8):
                        g, h = divmod(i, 2)
                        f0, f1 = (0, half) if h == 0 else (half,
                                                           srows * PADW)
                        qs[i % 3].dma_start(
                            x4[32 * g:32 * (g + 1), f0:f1],
                            xin.ap()[b, ssi, 32 * g:32 * (g + 1), f0:f1])
                else:
                    for g in range(G):
                        xqs[g].dma_start(
                            x4[32 * g:32 * (g + 1), :srows * PADW],
                            xin.ap()[b, ssi, 32 * g:32 * (g + 1),
                                     :srows * PADW])
                return x4

            def load_wt(b, j, nsplit=1):
                wt = wp.tile([128, NKX * 128], BF16, tag=f"wt{b}{j}")
                cuts = [0, 32, 64, 96, 128]
                step = 4 // nsplit
                for i in range(nsplit):
                    p0, p1 = cuts[i * step], cuts[(i + 1) * step]
                    qs[(b * NJ + j + i) % 3].dma_start(
                        wt[p0:p1, :], wts.ap()[b, j, p0:p1, :])
                return wt

            # weight chunk (0,0) first, split over queues: the very first
            # matmul block needs it; remaining chunks follow the first x4
            wtiles = {(0, 0): load_wt(0, 0, nsplit=2)}
            steps = [(b, ssi) for b in range(BLOC) for ssi in range(NSSE)]
            x4_next = load_x4(*steps[0], split=True)
            for bj in [(0, 1), (0, 2), (0, 3), (1, 0), (1, 1), (1, 2),
                       (1, 3)]:
                wtiles[bj] = load_wt(*bj)

            for si, (b, ssi) in enumerate(steps):
                y0, rows = SSPLAN[ssi]
                nrh = rows // 32
                x4 = x4_next
                if si + 1 < len(steps):
                    x4_next = load_x4(*steps[si + 1])
                x4a = x4[:]
                accs = [pp.tile([128, NFREE], F32, tag=f"acc{rh}{cs}",
                                name=f"acc{rh}{cs}_{si}")
                        for rh in range(nrh) for cs in range(CS)]
                # weight-block-outer order: each block feeds all accs
                # back-to-back; the BIR postprocess dedupes the repeated
                # prefetch Ldweights so each block is loaded once
                for j in range(NJ):
                    wt = wtiles[(b, j)]
                    for kx in range(NKX):
                        wblk = wt[:, kx * 128:(kx + 1) * 128]
                        for a, acc in enumerate(accs):
                            rh, cs = divmod(a, CS)
                            rhs = bass.AP(
                                x4a.tensor,
                                x4a.offset + (32 * rh + 4 * j) * PADW
                                + NCOL * cs + kx,
                                [list(x4a.ap[0]),
                                 [4 * PADW, NQ], [1, NCOL]])
                            nc.tensor.matmul(
                                acc[:], wblk, rhs,
                                start=(j == 0 and kx == 0),
                                stop=(j == NJ - 1 and kx == NKX - 1))
                # merge the 3 col strips of each row-half into one SBUF
                # tile so the output DMA has 384B-contiguous runs
                for rh in range(nrh):
                    ot = op.tile([128, NQ * W], F32, tag=f"ot{rh}",
                                 name=f"ot{rh}_{si}")
                    ota = ot[:]
                    for cs in range(CS):
                        acc = accs[rh * CS + cs]
                        acca = acc[:]
                        src = bass.AP(acca.tensor, acca.offset,
                                      [list(acca.ap[0]),
                                       [NCOL, NQ], [1, NCOL]])
                        dst = bass.AP(ota.tensor, ota.offset + NCOL * cs,
                                      [list(ota.ap[0]),
                                       [W, NQ], [1, NCOL]])
                        nc.vector.tensor_copy(dst, src)
                    last = si == len(steps) - 1
                    for dy in range(DY):
                        dma_dst = bass.AP(
                            yout_ap.tensor,
                            b * PD * H * W + (y0 + 32 * rh + dy) * W,
                            [[H * W, PD], [DY * W, NQ], [1, W]])
                        # outputs normally stay on the gpsimd queue (so
                        # they never delay x4 prefetches on sync/scalar);
                        # the final superstep has nothing left to prefetch
                        # -> rotate so the tail drains 3-way parallel
                        q = qs[dy % 3] if last else nc.gpsimd
                        q.dma_start(
                            dma_dst,
                            ot[32 * dy:32 * (dy + 1), :]
                            .rearrange("p (q c) -> p q c", c=W))
    return nc


_NC = None


def _get_nc():
    global _NC
    if _NC is None:
        _NC = _build_nc()
    return _NC


# ---------------------------------------------------------------------------
# Host side


def _gelu_exact(z):
    from math import erf
    return 0.5 * z * (1.0 + np.vectorize(erf)(z / np.sqrt(2.0)))


def _prepare_inputs(x, lk_filter, w1, b1, w2, b2):
    bf16 = mybir.dt.np(BF16)
    x = np.ascontiguousarray(np.asarray(x, dtype=np.float32))
    x1 = x[:, :PD]

    # dwc_proj on host (tiny): pool -> 1x1 -> exact GELU -> 1x1
    pooled = x1.mean(axis=(2, 3), dtype=np.float32)            # [B, 32]
    hid = _gelu_exact(pooled @ np.asarray(w1, np.float32).T
                      + np.asarray(b1, np.float32)).astype(np.float32)
    dyn_k = (hid @ np.asarray(w2, np.float32).T
             + np.asarray(b2, np.float32)).reshape(B, PD, SK, SK)

    # fold the per-sample depthwise 3x3 into the center of the 13x13 filter
    F = np.broadcast_to(np.asarray(lk_filter, np.float32),
                        (B, PD, PD, LK, LK)).copy()
    idx = np.arange(PD)
    ctr = PAD - SK // 2                                         # 5
    F[:, idx, idx, ctr:ctr + SK, ctr:ctr + SK] += dyn_k

    # weight blocks: wts[b, j, kx, g*32+ic, dy*32+oc] = F[b, oc, ic, 4j+g-dy, kx]
    wts = np.zeros((B, NJ, NKX, 128, 128), np.float32)
    for j in range(NJ):
        for g in range(G):
            for dy in range(DY):
                ky = 4 * j + g - dy
                if 0 <= ky < LK:
                    wts[:, j, :, g * PD:(g + 1) * PD,
                        dy * PD:(dy + 1) * PD] = \
                        F[:, :, :, ky, :].transpose(0, 3, 2, 1)
    # device layout [b, j, k, kx*128+m]: per-partition contiguous DMA runs
    wts_dev = np.ascontiguousarray(
        wts.astype(bf16).transpose(0, 1, 3, 2, 4)).reshape(
            B, NJ, 128, NKX * 128)

    xpad = np.zeros((B, PD, PADH, PADW), bf16)
    xpad[:, :, PAD:PAD + H, PAD:PAD + W] = x1.astype(bf16)
    # pre-replicate into the SBUF x4 layout: [b, ssi, g*32+ic, (s, c)]
    xrep = np.zeros((B, NSSE, G, PD, SROWS, PADW), bf16)
    for ssi, (y0, rows) in enumerate(SSPLAN):
        srows = rows + 9
        for g in range(G):
            xrep[:, ssi, g, :, :srows] = \
                xpad[:, :, y0 + g:y0 + g + srows, :]
    xrep = xrep.reshape(B, NSSE, 128, SROWS * PADW)

    in_maps = [{"xin": xrep[BLOC * c:BLOC * (c + 1)],
                "wts": wts_dev[BLOC * c:BLOC * (c + 1)]}
               for c in range(NCORES)]
    return x, in_maps


def _execute(in_maps, trace=False):
    nc = _get_nc()
    return run_bass_kernel_spmd(nc, in_maps, list(range(NCORES)), trace=trace)


def kernel(x, lk_filter, w1, b1, w2, b2):
    x, in_maps = _prepare_inputs(x, lk_filter, w1, b1, w2, b2)
    res = _execute(in_maps)
    out = np.empty((B, C, H, W), np.float32)
    for c in range(NCORES):
        out[BLOC * c:BLOC * (c + 1), :PD] = res.results[c]["yout"]
    out[:, PD:] = x[:, PD:]
    return out


# revision 34
# speedup vs baseline: 1.3092x; 1.0048x over previous
"""ConvolutionalAttention (training branch) for Trainium2, 8 NeuronCores.

The module computes, per sample b:
    out[:, :32]  = conv13x13(x1, lk_filter) + depthwise3x3(x1, dyn_k[b])
    out[:, 32:]  = x2            (pass-through)
where dyn_k[b] comes from a tiny MLP (pool -> 1x1 -> GELU -> 1x1) on x1.

Key transformation: conv is linear in the filter, so the per-sample dynamic
depthwise 3x3 kernel is folded host-side into the center of a per-sample
13x13 dense filter.  The device then runs ONE dense 32->32 13x13 conv per
sample.  Data-parallel over batch: 2 samples per core.

Device mapping (per core, per sample), v2:
  - conv as matmul with K = 128 = (4 row-shift replicas g) x (32 in-ch),
    M = 128 = (4 output rows dy) x (32 out-ch).
  - bf16 operands: same PE stream rate as fp32r, but half the DMA bytes
    and 2x faster LDWEIGHTS (fast-weight-load path).  Accumulation stays
    fp32 in PSUM; observed end-to-end rel err ~2e-3 vs 2e-2 budget.
  - supersteps of 64 output rows: 6 PSUM accumulators [128, 512], each
    covering 32 rows x 64 cols via an overlapped rhs access pattern
    (8 quads x 64 cols).  52 weight blocks (4 ky'-chunks x 13 kx) feed
    6 back-to-back matmuls each, so walrus's redundant-LDWEIGHTS elision
    drops 5 of every 6 weight loads.
  - input/weight DMAs spread over 4 engine queues to shrink the initial
    transfer-limited head; dummy warm-up matmuls keep the PE HAM clock
    gate at 2.4 GHz by the time the real stream starts.
"""

import json

import numpy as np

import concourse.bass as bass
import concourse.mybir as mybir
import concourse.tile as tile
from concourse.bass_utils import run_bass_kernel_spmd

# ---------------------------------------------------------------------------
# Problem constants (hardcoded; kernel.py must be self-contained)
B, C, H, W = 16, 64, 192, 192
PD, SK, LK = 32, 3, 13
PAD = LK // 2                      # 6
NCORES = 8
BLOC = B // NCORES                 # 2 samples per core
PADW = W + 2 * PAD                 # 204
PADH = H + 2 * PAD                 # 204
NJ, G, DY = 4, 4, 4                # ky' chunks, row-shift replicas, rows/quad
NKX = LK                           # 13 kx shifts
SSPLAN = [(0, 32), (32, 64), (96, 64), (160, 32)]  # (y0, rows) supersteps
NSSE = len(SSPLAN)                 # small first superstep -> short DMA head;
                                   # small last superstep -> short drain tail
CS = 3                             # 64-col strips per superstep
NQ = 8                             # quads per accumulator
NCOL = 64                          # cols per strip
SROWS = 73                         # max x4 rows per superstep (per g)
NFREE = NQ * NCOL                  # 512 matmul moving free dim (1 PSUM bank)
NWARM = 18                         # dummy warm-up matmuls (N=256, ~3.8us)
NWFREE = 256                       # warm-up matmul free dim
SLOTBASE = [0, 1, 3, 5]            # output scratch slot per superstep
NSLOT = 6                          # rh-slots per sample (1+2+2+1)
F32 = mybir.dt.float32
BF16 = mybir.dt.bfloat16

# ---------------------------------------------------------------------------
# Workaround: the walrus_driver in this container rejects instructions with
# more than one sync-wait command.  Post-process the BIR JSON, moving excess
# waits onto single-wait NoOps inserted right before the offending
# instruction (same engine => executes first, semantics preserved).
_orig_to_json_bytes = bass.Bass.to_json_bytes


def _split_multi_waits(m):
    import json as _json
    for f in m.get("functions", []):
        for blk in f.get("blocks", []):
            out = []
            changed = False
            last_ldw_sig = [None]
            for inst in blk.get("instructions", []):
                si = inst.get("sync_info")
                waits = (si or {}).get("on_wait") or []
                # strip sync waits off Ldweights onto NoOps so the dedup
                # below can't drop a load-bearing wait
                keep = 0 if inst["opcode"] == "Ldweights" else 1
                if len(waits) > keep:
                    changed = True
                    for k, wcond in enumerate(waits[:len(waits) - keep]):
                        out.append({
                            "debug": inst.get("debug"),
                            "engine": inst["engine"],
                            "ins": [], "outs": [],
                            "name": f"{inst['name']}.sw{k}",
                            "opcode": "NoOp",
                            "sync_info": {"on_update": [], "on_wait": [wcond]},
                            "text_hint": "split_wait",
                        })
                    si["on_wait"] = waits[len(waits) - keep:]
                # dedup: the bf16 lowering emits one prefetch Ldweights per
                # Matmult (the Matmults have ldweights=false).  Consecutive
                # identical Ldweights are idempotent -> drop repeats so each
                # weight block is loaded once per 6-matmul group.
                if inst["engine"] == "PE":
                    if inst["opcode"] == "Ldweights":
                        sig = _json.dumps(
                            [inst.get("ins"), inst.get("tile_position"),
                             inst.get("perf_mode"),
                             inst.get("is_transpose")], sort_keys=True)
                        if sig == last_ldw_sig[0]:
                            changed = True
                            ups = (si or {}).get("on_update") or []
                            if ups:
                                out.append({
                                    "debug": inst.get("debug"),
                                    "engine": inst["engine"],
                                    "ins": [], "outs": [],
                                    "name": f"{inst['name']}.dup",
                                    "opcode": "NoOp",
                                    "sync_info": {"on_update": ups,
                                                  "on_wait": []},
                                    "text_hint": "dedup_ldw",
                                })
                            continue
                        last_ldw_sig[0] = sig
                    elif inst["opcode"] not in ("Matmult", "NoOp",
                                                "EventSemaphore"):
                        last_ldw_sig[0] = None
                out.append(inst)
            if changed:
                blk["instructions"] = out
    return m


def _to_json_bytes_split(self, *a, **kw):
    data = _orig_to_json_bytes(self, *a, **kw)
    return json.dumps(_split_multi_waits(json.loads(data))).encode()


def _install_patch():
    if bass.Bass.to_json_bytes is not _to_json_bytes_split:
        bass.Bass.to_json_bytes = _to_json_bytes_split
    # NOTE: walrus's --enable-ldw-opt is left at its default (false): the
    # bf16 path lowers each matmul to a standalone prefetch Ldweights +
    # self-loading Matmult, and walrus's ldw-opt rejects standalone
    # InstLdweights outright.  The PE's 64-deep reorder window pulls the
    # prefetch Ldweights ahead of in-flight matmuls instead.


# ---------------------------------------------------------------------------
# Device kernel


def _build_nc():
    _install_patch()
    nc = bass.Bass()
    # xin is pre-replicated host-side into the exact SBUF x4 layout
    # (partition = g*32+ic, free = (s, c)) so every DMA run is a full
    # contiguous per-partition read
    xin = nc.declare_dram_parameter("xin", [BLOC, NSSE, 128, SROWS * PADW],
                                    BF16, isOutput=False)
    wts = nc.declare_dram_parameter("wts", [BLOC, NJ, 128, NKX * 128], BF16,
                                    isOutput=False)
    # output goes to a contiguous bf16 scratch layout (one [128, 1536]
    # dump per 32-row half); the host reassembles — 3KB DMA runs instead
    # of 768B row-scatters, half the bytes
    yout = nc.declare_dram_parameter("yout", [BLOC, NSLOT, 128, NQ * W],
                                     BF16, isOutput=True)

    with tile.TileContext(nc) as tc:
        with tc.tile_pool(name="wp", bufs=1) as wp, \
             tc.tile_pool(name="xp", bufs=2) as xp, \
             tc.tile_pool(name="sp", bufs=1) as sp, \
             tc.tile_pool(name="pp", bufs=1, space="PSUM") as pp, \
             tc.tile_pool(name="op", bufs=2) as op:

            # ---- warm-up: keep PE busy during the initial DMA head so the
            # HAM clock gate reaches 2.4 GHz before the real stream starts
            warm_w = sp.tile([128, 128], BF16, tag="warmw")
            warm_x = sp.tile([128, NWFREE], BF16, tag="warmx")
            nc.vector.memset(warm_w[:], 0.0)
            nc.vector.memset(warm_x[:], 0.0)
            warm_acc = pp.tile([128, NWFREE], F32, tag="warm")
            for _ in range(NWARM):
                nc.tensor.matmul(warm_acc[:], warm_w[:], warm_x[:],
                                 start=True, stop=True)

            # input x4 loads: one 32-partition-aligned DMA per row-shift
            # replica g, spread over the 3 DMA-capable queues (partition
            # slices must stay 32-aligned: unaligned chunks transfer ~3x
            # slower and their SBUF writes contend with PE reads)
            qs = [nc.sync, nc.scalar, nc.gpsimd]
            xqs = [nc.sync, nc.scalar, nc.gpsimd, nc.scalar]

            def load_x4(b, ssi, split=False):
                rows = SSPLAN[ssi][1]
                srows = rows + 9
                x4 = xp.tile([128, SROWS * PADW + 16], BF16, tag="x4")
                if split:
                    # first load: 8 half-row DMAs balanced over the 3
                    # queues so the head transfer finishes sooner
                    half = (srows // 2) * PADW
                    for i in range(8):
                        g, h = divmod(i, 2)
                        f0, f1 = (0, half) if h == 0 else (half,
                                                           srows * PADW)
                        qs[i % 3].dma_start(
                            x4[32 * g:32 * (g + 1), f0:f1],
                            xin.ap()[b, ssi, 32 * g:32 * (g + 1), f0:f1])
                else:
                    for g in range(G):
                        xqs[g].dma_start(
                            x4[32 * g:32 * (g + 1), :srows * PADW],
                            xin.ap()[b, ssi, 32 * g:32 * (g + 1),
                                     :srows * PADW])
                return x4

            def load_wt(b, j, nsplit=1):
                wt = wp.tile([128, NKX * 128], BF16, tag=f"wt{b}{j}")
                cuts = [0, 32, 64, 96, 128]
                step = 4 // nsplit
                for i in range(nsplit):
                    p0, p1 = cuts[i * step], cuts[(i + 1) * step]
                    qs[(b * NJ + j + i) % 3].dma_start(
                        wt[p0:p1, :], wts.ap()[b, j, p0:p1, :])
                return wt

            # weight chunk (0,0) first, split over queues: the very first
            # matmul block needs it; remaining chunks follow the first x4
            wtiles = {(0, 0): load_wt(0, 0, nsplit=2)}
            steps = [(b, ssi) for b in range(BLOC) for ssi in range(NSSE)]
            x4_next = load_x4(*steps[0], split=True)
            for bj in [(0, 1), (0, 2), (0, 3), (1, 0), (1, 1), (1, 2),
                       (1, 3)]:
                wtiles[bj] = load_wt(*bj)

            for si, (b, ssi) in enumerate(steps):
                y0, rows = SSPLAN[ssi]
                nrh = rows // 32
                x4 = x4_next
                if si + 1 < len(steps):
                    x4_next = load_x4(*steps[si + 1])
                x4a = x4[:]
                accs = [pp.tile([128, NFREE], F32, tag=f"acc{rh}{cs}",
                                name=f"acc{rh}{cs}_{si}")
                        for rh in range(nrh) for cs in range(CS)]
                # weight-block-outer order: each block feeds all accs
                # back-to-back; the BIR postprocess dedupes the repeated
                # prefetch Ldweights so each block is loaded once
                for j in range(NJ):
                    wt = wtiles[(b, j)]
                    for kx in range(NKX):
                        wblk = wt[:, kx * 128:(kx + 1) * 128]
                        for a, acc in enumerate(accs):
                            rh, cs = divmod(a, CS)
                            rhs = bass.AP(
                                x4a.tensor,
                                x4a.offset + (32 * rh + 4 * j) * PADW
                                + NCOL * cs + kx,
                                [list(x4a.ap[0]),
                                 [4 * PADW, NQ], [1, NCOL]])
                            nc.tensor.matmul(
                                acc[:], wblk, rhs,
                                start=(j == 0 and kx == 0),
                                stop=(j == NJ - 1 and kx == NKX - 1))
                # merge the 3 col strips of each row-half into one bf16
                # SBUF tile, then dump it contiguously to the scratch
                # output (the host reassembles the [dy, oc, q, c] layout)
                last = si == len(steps) - 1
                for rh in range(nrh):
                    ot = op.tile([128, NQ * W], BF16, tag=f"ot{rh}",
                                 name=f"ot{rh}_{si}")
                    ota = ot[:]
                    for cs in range(CS):
                        acc = accs[rh * CS + cs]
                        acca = acc[:]
                        src = bass.AP(acca.tensor, acca.offset,
                                      [list(acca.ap[0]),
                                       [NCOL, NQ], [1, NCOL]])
                        dst = bass.AP(ota.tensor, ota.offset + NCOL * cs,
                                      [list(ota.ap[0]),
                                       [W, NQ], [1, NCOL]])
                        nc.vector.tensor_copy(dst, src)
                    slot = SLOTBASE[ssi] + rh
                    if last:
                        # nothing left to prefetch: split the final dump
                        # across the 3 queues so the tail drains parallel
                        for i, (p0, p1) in enumerate([(0, 64), (64, 96),
                                                      (96, 128)]):
                            qs[i].dma_start(
                                yout.ap()[b, slot, p0:p1, :],
                                ot[p0:p1, :])
                    else:
                        # outputs stay on gpsimd so they never delay x4
                        # prefetches on sync/scalar
                        nc.gpsimd.dma_start(yout.ap()[b, slot], ota)
    return nc


_NC = None


def _get_nc():
    global _NC
    if _NC is None:
        _NC = _build_nc()
    return _NC


# ---------------------------------------------------------------------------
# Host side


def _gelu_exact(z):
    from math import erf
    return 0.5 * z * (1.0 + np.vectorize(erf)(z / np.sqrt(2.0)))


def _prepare_inputs(x, lk_filter, w1, b1, w2, b2):
    bf16 = mybir.dt.np(BF16)
    x = np.ascontiguousarray(np.asarray(x, dtype=np.float32))
    x1 = x[:, :PD]

    # dwc_proj on host (tiny): pool -> 1x1 -> exact GELU -> 1x1
    pooled = x1.mean(axis=(2, 3), dtype=np.float32)            # [B, 32]
    hid = _gelu_exact(pooled @ np.asarray(w1, np.float32).T
                      + np.asarray(b1, np.float32)).astype(np.float32)
    dyn_k = (hid @ np.asarray(w2, np.float32).T
             + np.asarray(b2, np.float32)).reshape(B, PD, SK, SK)

    # fold the per-sample depthwise 3x3 into the center of the 13x13 filter
    F = np.broadcast_to(np.asarray(lk_filter, np.float32),
                        (B, PD, PD, LK, LK)).copy()
    idx = np.arange(PD)
    ctr = PAD - SK // 2                                         # 5
    F[:, idx, idx, ctr:ctr + SK, ctr:ctr + SK] += dyn_k

    # weight blocks: wts[b, j, kx, g*32+ic, dy*32+oc] = F[b, oc, ic, 4j+g-dy, kx]
    wts = np.zeros((B, NJ, NKX, 128, 128), np.float32)
    for j in range(NJ):
        for g in range(G):
            for dy in range(DY):
                ky = 4 * j + g - dy
                if 0 <= ky < LK:
                    wts[:, j, :, g * PD:(g + 1) * PD,
                        dy * PD:(dy + 1) * PD] = \
                        F[:, :, :, ky, :].transpose(0, 3, 2, 1)
    # device layout [b, j, k, kx*128+m]: per-partition contiguous DMA runs
    wts_dev = np.ascontiguousarray(
        wts.astype(bf16).transpose(0, 1, 3, 2, 4)).reshape(
            B, NJ, 128, NKX * 128)

    xpad = np.zeros((B, PD, PADH, PADW), bf16)
    xpad[:, :, PAD:PAD + H, PAD:PAD + W] = x1.astype(bf16)
    # pre-replicate into the SBUF x4 layout: [b, ssi, g*32+ic, (s, c)]
    xrep = np.zeros((B, NSSE, G, PD, SROWS, PADW), bf16)
    for ssi, (y0, rows) in enumerate(SSPLAN):
        srows = rows + 9
        for g in range(G):
            xrep[:, ssi, g, :, :srows] = \
                xpad[:, :, y0 + g:y0 + g + srows, :]
    xrep = xrep.reshape(B, NSSE, 128, SROWS * PADW)

    in_maps = [{"xin": xrep[BLOC * c:BLOC * (c + 1)],
                "wts": wts_dev[BLOC * c:BLOC * (c + 1)]}
               for c in range(NCORES)]
    return x, in_maps


def _execute(in_maps, trace=False):
    nc = _get_nc()
    return run_bass_kernel_spmd(nc, in_maps, list(range(NCORES)), trace=trace)


def kernel(x, lk_filter, w1, b1, w2, b2):
    x, in_maps = _prepare_inputs(x, lk_filter, w1, b1, w2, b2)
    res = _execute(in_maps)
    out = np.empty((B, C, H, W), np.float32)
    for c in range(NCORES):
        # scratch [BLOC, NSLOT, 128, NQ*W] -> [b, oc, y, x]
        scr = res.results[c]["yout"].astype(np.float32).reshape(
            BLOC, NSLOT, DY, PD, NQ, W)
        for ssi, (y0, rows) in enumerate(SSPLAN):
            for rh in range(rows // 32):
                slot = SLOTBASE[ssi] + rh
                # rows y0+32rh+4q+dy <- [dy, oc, q, c]
                blk = scr[:, slot].transpose(0, 2, 3, 1, 4).reshape(
                    BLOC, PD, 32, W)
                out[BLOC * c:BLOC * (c + 1), :PD,
                    y0 + 32 * rh:y0 + 32 * rh + 32] = blk
    out[:, PD:] = x[:, PD:]
    return out
